# revision 3
# baseline (speedup 1.0000x reference)
"""Multi-head attention (B=2, N=2048, D=768, H=12, Dh=64) on 8 TRN2 NeuronCores.

Sharding: head-parallel Megatron-style. Core c handles batch b=c//4 and heads
[3*(c%4), 3*(c%4)+3). Each core projects q/k/v for its 3 heads (column-sliced
Wq/Wkv), runs softmax(q k^T/8) v on-chip, and computes a partial out-projection
against its row-slice of Wproj. Host sums the 4 partials per batch + bias.

On-chip layout: sources are host-pre-transposed so projections are natural
matmuls. Scores are computed transposed (S^T: k on partitions, q free) so the
attn@v matmul consumes exp(S^T) directly as the streaming operand with
lhsT = [v | ones]; the ones column yields the softmax denominator for free.
"""

import os
import sys

sys.path.insert(0, "/opt/trn_rl_repo")

from contextlib import ExitStack

import ml_dtypes
import numpy as np

import concourse.bass as bass
import concourse.bacc as bacc
import concourse.tile as tile
from concourse import mybir
from concourse.bass_utils import run_bass_kernel_spmd

bf16 = ml_dtypes.bfloat16
F32 = mybir.dt.float32
BF16 = mybir.dt.bfloat16
EXP = mybir.ActivationFunctionType.Exp

P = 128          # partitions
NQ = 2048        # query length (per batch)
NKV = 2048       # kv length
D = 768          # model dim
DH = 64          # head dim
HL = 3           # heads per core
DL = HL * DH     # local projected dim (192)
KB = D // P      # contraction blocks for projections (6)
NKB = NKV // P   # k-index blocks (16)
QC = 1024        # q chunk for the attention inner loop
NQC = NQ // QC   # 2
SCALE = DH ** -0.5

_CACHE: dict = {}
LAST_RESULTS = None


def _build_program() -> bass.Bass:
    nc = bacc.Bacc("TRN2", target_bir_lowering=False)

    qsT = nc.dram_tensor("qsT", [D, NQ], BF16, kind="ExternalInput")
    kvT = nc.dram_tensor("kvT", [D, NKV], BF16, kind="ExternalInput")
    wq = nc.dram_tensor("wq", [D, DL], BF16, kind="ExternalInput")
    wk = nc.dram_tensor("wk", [D, DL], BF16, kind="ExternalInput")
    wv = nc.dram_tensor("wv", [D, DL], BF16, kind="ExternalInput")
    wp = nc.dram_tensor("wp", [DL, D], BF16, kind="ExternalInput")
    out = nc.dram_tensor("out", [NQ, D], BF16, kind="ExternalOutput")

    with tile.TileContext(nc) as tc, ExitStack() as ctx:
        sb_src = ctx.enter_context(tc.tile_pool(name="src", bufs=KB))
        sb_w = ctx.enter_context(tc.tile_pool(name="wts", bufs=KB))
        sb_p = ctx.enter_context(tc.tile_pool(name="persist", bufs=1))
        sb_es = ctx.enter_context(tc.tile_pool(name="es", bufs=4))
        sb_sm = ctx.enter_context(tc.tile_pool(name="small", bufs=2))
        sb_ob = ctx.enter_context(tc.tile_pool(name="outsb", bufs=3))

        # ---- DMA inputs ----
        qsT_sb, kvT_sb, wq_sb, wk_sb, wv_sb = [], [], [], [], []
        for kb in range(KB):
            t = sb_src.tile([P, NQ], BF16, tag="qsT")
            nc.sync.dma_start(t[:], qsT[kb * P : (kb + 1) * P, :])
            qsT_sb.append(t)
            t = sb_src.tile([P, NKV], BF16, tag="kvT")
            nc.sync.dma_start(t[:], kvT[kb * P : (kb + 1) * P, :])
            kvT_sb.append(t)
            for lst, dram, tag in ((wq_sb, wq, "wq"), (wk_sb, wk, "wk"), (wv_sb, wv, "wv")):
                t = sb_w.tile([P, DL], BF16, tag=tag)
                nc.sync.dma_start(t[:], dram[kb * P : (kb + 1) * P, :])
                lst.append(t)
        wp01 = sb_p.tile([P, D], BF16, tag="wp01")
        nc.sync.dma_start(wp01[:], wp[0:P, :])
        wp2 = sb_p.tile([DH, D], BF16, tag="wp2")
        nc.sync.dma_start(wp2[:], wp[P : P + DH, :])

        # ---- persistent intermediates ----
        qT01 = sb_p.tile([P, NQ], BF16, tag="qT01")   # q^T heads 0,1 (d on partitions)
        kT01 = sb_p.tile([P, NKV], BF16, tag="kT01")
        qT2 = sb_p.tile([DH, NQ], BF16, tag="qT2")    # q^T head 2
        kT2 = sb_p.tile([DH, NKV], BF16, tag="kT2")
        vA = sb_p.tile([P, HL * NKB * 65], BF16, tag="vA")  # per (h, kb): [v(64) | ones]
        X01 = sb_p.tile([P, NQ], BF16, tag="X01")     # normalized x^T heads 0,1
        X2 = sb_p.tile([DH, NQ], BF16, tag="X2")
        ones = sb_p.tile([1, DH], BF16, tag="ones")
        nc.vector.memset(ones[:], 1.0)
        nc.vector.memset(vA[:], 1.0)  # ones columns; v evacs overwrite the rest

        # ================= Phase 1: projections =================
        with tc.tile_pool(name="ppj", bufs=3, space="PSUM") as ppj, \
             tc.tile_pool(name="pv", bufs=2, space="PSUM") as pv:
            # q^T / k^T for heads 0,1 (full 128-column weight blocks)
            for w_sb, dst in ((wq_sb, qT01), (wk_sb, kT01)):
                for half in range(NQC):
                    ps = ppj.tile([P, QC], F32, tag="ppj")
                    for kb in range(KB):
                        for j in range(QC // 512):
                            nc.tensor.matmul(
                                ps[:, j * 512 : (j + 1) * 512],
                                w_sb[kb][:, 0:P],
                                qsT_sb[kb][:, half * QC + j * 512 : half * QC + (j + 1) * 512]
                                if w_sb is wq_sb
                                else kvT_sb[kb][:, half * QC + j * 512 : half * QC + (j + 1) * 512],
                                start=(kb == 0),
                                stop=(kb == KB - 1),
                            )
                    nc.vector.tensor_copy(dst[:, half * QC : (half + 1) * QC], ps[:])
            # q^T / k^T head 2 — col-tiled pair (q at psum 0:64, k at 64:128)
            for half in range(NQC):
                ps = ppj.tile([P, QC], F32, tag="ppj")
                for kb in range(KB):
                    for j in range(QC // 512):
                        sl = slice(j * 512, (j + 1) * 512)
                        src_sl = slice(half * QC + j * 512, half * QC + (j + 1) * 512)
                        nc.tensor.matmul(
                            ps[0:DH, sl], wq_sb[kb][:, P:DL], qsT_sb[kb][:, src_sl],
                            start=(kb == 0), stop=(kb == KB - 1),
                        )
                        nc.tensor.matmul(
                            ps[DH:P, sl], wk_sb[kb][:, P:DL], kvT_sb[kb][:, src_sl],
                            start=(kb == 0), stop=(kb == KB - 1),
                        )
                nc.vector.tensor_copy(qT2[:, half * QC : (half + 1) * QC], ps[0:DH, :])
                nc.vector.tensor_copy(kT2[:, half * QC : (half + 1) * QC], ps[DH:P, :])
            # v projection: (k-idx, d_local), scattered into vA with ones columns
            vA_view = vA[:].rearrange("p (h k c) -> p h k c", h=HL, k=NKB)
            for m in range(NKB):
                ps = pv.tile([P, DL], F32, tag="pv")
                for kb in range(KB):
                    nc.tensor.matmul(
                        ps[:], kvT_sb[kb][:, m * P : (m + 1) * P], wv_sb[kb][:],
                        start=(kb == 0), stop=(kb == KB - 1),
                    )
                nc.vector.tensor_copy(
                    vA_view[:, :, m, 0:DH],
                    ps[:].rearrange("p (h d) -> p h d", h=HL),
                )

        # ================= Phase 2: attention =================
        with tc.tile_pool(name="ps_s", bufs=2, space="PSUM") as ps_s, \
             tc.tile_pool(name="ps_x", bufs=2, space="PSUM") as ps_x:
            for h in range(HL):
                if h < 2:
                    kT_h = kT01[h * DH : (h + 1) * DH, :]
                    qT_h = qT01[h * DH : (h + 1) * DH, :]
                    X_h = X01[h * DH : (h + 1) * DH, :]
                else:
                    kT_h, qT_h, X_h = kT2[:], qT2[:], X2[:]
                for qc in range(NQC):
                    xps = ps_x.tile([65, QC], F32, tag="xps")
                    for kb in range(NKB):
                        sc = ps_s.tile([P, QC], F32, tag="sc")
                        for j in range(QC // 512):
                            nc.tensor.matmul(
                                sc[:, j * 512 : (j + 1) * 512],
                                kT_h[:, kb * P : (kb + 1) * P],
                                qT_h[:, qc * QC + j * 512 : qc * QC + (j + 1) * 512],
                                start=True, stop=True,
                            )
                        es = sb_es.tile([P, QC], BF16, tag="es")
                        nc.scalar.activation(es[:], sc[:], EXP, scale=SCALE)
                        for j in range(QC // 512):
                            sl = slice(j * 512, (j + 1) * 512)
                            nc.tensor.matmul(
                                xps[:, sl],
                                vA[:, (h * NKB + kb) * 65 : (h * NKB + kb + 1) * 65],
                                es[:, sl],
                                start=(kb == 0), stop=(kb == NKB - 1),
                            )
                    # softmax denominator -> reciprocal -> broadcast -> normalize
                    dn = sb_sm.tile([1, QC], F32, tag="dn")
                    nc.vector.tensor_copy(dn[:], xps[64:65, :])
                    rc = sb_sm.tile([1, QC], F32, tag="rc")
                    nc.vector.reciprocal_approx_fast(rc[:], dn[:])
                    rcb = sb_sm.tile([1, QC], BF16, tag="rcb")
                    nc.vector.tensor_copy(rcb[:], rc[:])
                    bc = ps_x.tile([65, QC], F32, tag="xps")
                    for j in range(QC // 512):
                        sl = slice(j * 512, (j + 1) * 512)
                        nc.tensor.matmul(bc[0:DH, sl], ones[:], rcb[:, sl], start=True, stop=True)
                    bcs = sb_sm.tile([DH, QC], BF16, tag="bcs")
                    nc.vector.tensor_copy(bcs[:], bc[0:DH, :])
                    nc.vector.tensor_mul(
                        X_h[:, qc * QC : (qc + 1) * QC], xps[0:DH, :], bcs[:]
                    )

        # ================= Phase 3: output projection =================
        with tc.tile_pool(name="ps_o", bufs=3, space="PSUM") as ps_o:
            for m in range(NKB):
                po = ps_o.tile([P, D], F32, tag="po")
                for j, n in ((0, 512), (512, 256)):
                    nc.tensor.matmul(
                        po[:, j : j + n], X01[:, m * P : (m + 1) * P], wp01[:, j : j + n],
                        start=True, stop=False,
                    )
                    nc.tensor.matmul(
                        po[:, j : j + n], X2[:, m * P : (m + 1) * P], wp2[:, j : j + n],
                        start=False, stop=True,
                    )
                ob = sb_ob.tile([P, D], BF16, tag="ob")
                nc.vector.tensor_copy(ob[:], po[:])
                nc.sync.dma_start(out[m * P : (m + 1) * P, :], ob[:])

    nc.compile()
    return nc


def _get_nc() -> bass.Bass:
    if "nc" not in _CACHE:
        _CACHE["nc"] = _build_program()
    return _CACHE["nc"]


def kernel(**inputs) -> np.ndarray:
    global LAST_RESULTS
    qs = np.asarray(inputs["query_source"], dtype=np.float32)
    kv = np.asarray(inputs["kv_source"], dtype=np.float32)
    Wq = np.asarray(inputs["Wq"], dtype=np.float32)
    Wkv = np.asarray(inputs["Wkv"], dtype=np.float32)
    Wp = np.asarray(inputs["Wproj"], dtype=np.float32)
    bp = np.asarray(inputs["bproj"], dtype=np.float32)

    nc = _get_nc()
    in_maps = []
    for c in range(8):
        b = c // 4
        c0 = (c % 4) * DL
        in_maps.append(
            {
                "qsT": np.ascontiguousarray(qs[b].T).astype(bf16),
                "kvT": np.ascontiguousarray(kv[b].T).astype(bf16),
                "wq": Wq[:, c0 : c0 + DL].astype(bf16),
                "wk": Wkv[:, c0 : c0 + DL].astype(bf16),
                "wv": Wkv[:, D + c0 : D + c0 + DL].astype(bf16),
                "wp": Wp[c0 : c0 + DL, :].astype(bf16),
            }
        )

    trace = bool(int(os.environ.get("KERNEL_TRACE", "0")))
    res = run_bass_kernel_spmd(nc, in_maps, list(range(8)), trace=trace)
    LAST_RESULTS = res

    out = np.tile(bp.astype(np.float32), (2, NQ, 1))
    for c in range(8):
        out[c // 4] += res.results[c]["out"].astype(np.float32)
    return out
